# revision 2
# baseline (speedup 1.0000x reference)
# Trainium2 Bass kernel v2 for the LeNet-C3 sparse-connection conv problem.
#
# Math: VALID 2D conv, input [32, 512, 512, 6] f32, dense kernel [5,5,6,16]
# (assembled from the sparse C3 connection tables), + bias -> [32, 508, 508, 16].
#
# v2 strategy (vs v1's single full-array matmul stream):
#   - Two concurrent PE row-strip lanes via 32-granular array tiling:
#     lane A = partitions 0-59, lane B = partitions 64-123. Each lane runs
#     K=60, M=96 matmuls (6 output pixels x 16 channels per group, one
#     filter row dy per matmul, 5 accumulating matmuls per group). The two
#     strips stream concurrently -> ~2x effective PE throughput vs one
#     K=72/M=128 stream, and the dy shift stays in the moving free dim so
#     no shifted data copies are needed.
#   - Input is pre-windowed on host: xg[img, group, 60, 512] bf16 where
#     xg[i,g,f,y] = x[i, y, 36g+f]. Device DMAs read 1KB-contiguous runs.
#   - Output staged in SBUF as bf16 per 8-group chunk and written with
#     ~1.5KB-contiguous bursts; host casts back to f32 (rel-err budget 2e-2,
#     bf16 rounding adds ~0.4% worst-case elementwise).
#   - Bias-add alternates ScalarE (lane A) / VectorE (lane B).

import numpy as np
import ml_dtypes

BATCH, H, W, CIN, COUT, FS = 32, 512, 512, 6, 16, 5
N_CORES = 8
IMGS_PER_CORE = BATCH // N_CORES  # 4
HO = WO = H - FS + 1  # 508
FLAT = W * CIN  # 3072
FLAT_PAD = 3096  # last group window ends at 36*84+60 = 3084
P = 6  # output pixels per group
NG = 85  # ceil(508/6); last group has 4 valid pixels
KW = 6 * (P + 4)  # 60: window partitions per group
M = 16 * P  # 96 live psum partitions per group
MP = 128  # weight columns padded to 128 so the compiler enables FWL
CHUNK = 8  # global groups per output chunk
_CACHE = {}


def _dense_kernel_np(weights3, weights4, weights4_4, weights6):
    """Numpy port of reference._dense_kernel: [5,5,6,16] dense conv kernel."""
    f = weights3.shape[0]
    Wd = np.zeros((f, f, CIN, COUT), dtype=np.float32)
    for i in range(6):
        for m in range(3):
            Wd[:, :, (i + m) % 6, i] = weights3[:, :, m, i]
    for k in range(6):
        for m in range(4):
            Wd[:, :, (k + m) % 6, 6 + k] = weights4[:, :, m, k]
    for k in range(3):
        for m, off in enumerate((0, 1, 3, 4)):
            Wd[:, :, (k + off) % 6, 12 + k] = weights4_4[:, :, m, k]
    Wd[:, :, :, 15] = weights6[:, :, :, 0]
    return Wd


def _build_wbands(Wd):
    """[FS, KW, M]: banded weights. W[dy, 6(j+dx)+c, 16j+co] = Wd[dy,dx,c,co]."""
    wb = np.zeros((FS, KW, MP), dtype=np.float32)
    for dy in range(FS):
        for j in range(P):
            for dx in range(FS):
                for c in range(CIN):
                    wb[dy, 6 * (j + dx) + c, 16 * j:16 * j + 16] = \
                        Wd[dy, dx, c, :]
    return wb


def _split_excess_waits(nc, max_waits=1):
    """This image's walrus rejects instructions carrying more than one sem
    wait ("Too many sync wait commands" in setupSyncWait). Tile freely
    attaches several waits to one instruction. Hoist the extras onto
    nofuse NOPs inserted just before, on the same engine — identical
    semantics (all waits retired before the instruction issues)."""
    import concourse.mybir as mybir

    for f in nc.m.functions:
        for bb in f.blocks:
            new_list = []
            changed = False
            for inst in bb.instructions:
                si = inst.sync_info
                waits = list(si.on_wait) if si and si.on_wait else []
                if len(waits) > max_waits:
                    changed = True
                    for k, w in enumerate(waits[max_waits:]):
                        nop = mybir.InstNoOp(
                            name=f"{inst.name}-wsplit{k}",
                            sync_info=mybir.SyncInfo(on_wait=[w], on_update=[]),
                            bass_nofuse=True,
                            engine=inst.engine,
                        )
                        new_list.append(nop)
                    si.on_wait = waits[:max_waits]
                new_list.append(inst)
            if changed:
                bb.instructions = new_list


def _chunk_layout():
    """Per chunk: (global group list, lane-A indices, lane-B indices)."""
    chunks = []
    for c0 in range(0, NG, CHUNK):
        gl = list(range(c0, min(c0 + CHUNK, NG)))
        la = [g // 2 for g in gl if g % 2 == 0]
        lb = [g // 2 for g in gl if g % 2 == 1]
        chunks.append((gl, la, lb))
    return chunks


def _build_nc(n_imgs=IMGS_PER_CORE, split_waits=True):
    import concourse.bass as bass
    import concourse.mybir as mybir
    from concourse.tile import TileContext

    NA = (NG + 1) // 2  # 43 lane-A groups (even g)
    NB = NG // 2        # 42 lane-B groups (odd g)

    nc = bass.Bass(trn_type="TRN2")
    xa = nc.dram_tensor("xa", (n_imgs, NA, KW, H), mybir.dt.bfloat16,
                        kind="ExternalInput")
    xb = nc.dram_tensor("xb", (n_imgs, NB, KW, H), mybir.dt.bfloat16,
                        kind="ExternalInput")
    w = nc.dram_tensor("w", (124, FS * MP), mybir.dt.bfloat16,
                       kind="ExternalInput")
    b = nc.dram_tensor("b", (M, 1), mybir.dt.float32, kind="ExternalInput")
    # transposed output layout [img, x, c, y(pad 512)]: DMA partition dim is
    # (x c) (merges to stride-512), free dims (group, y) with y contiguous —
    # fits the 3-dim DMA AP limit with ~1KB bursts. Host transposes back.
    out = nc.dram_tensor("out", (n_imgs, P * NG, COUT, H), mybir.dt.bfloat16,
                         kind="ExternalOutput")

    chunks = _chunk_layout()

    with TileContext(nc) as tc:
        with tc.tile_pool(name="const", bufs=1) as cpool, \
             tc.tile_pool(name="xin", bufs=3) as xpool, \
             tc.tile_pool(name="stage", bufs=3) as spool, \
             tc.tile_pool(name="ps", bufs=7, space="PSUM") as ppool:
            wt = cpool.tile([124, FS * MP], mybir.dt.bfloat16, name="wt")
            nc.sync.dma_start(out=wt[:, :], in_=w[:, :])
            bt = cpool.tile([M, 1], mybir.dt.float32, name="bt")
            nc.sync.dma_start(out=bt[:, :], in_=b[:, :])

            # warm-up matmuls: keep the PE busy through the HAM activity
            # window while the first input chunks are still in flight, so
            # real matmuls start at 2.4 GHz instead of 1.2.
            warm = ppool.tile([MP, 480], mybir.dt.float32, name="warm",
                              tag="warm", bufs=1)
            for _ in range(16):
                nc.tensor.matmul(warm[:, :], wt[0:KW, 0:MP],
                                 wt[0:KW, 0:480], start=True, stop=True)

            for n in range(n_imgs):
                for gl, la, lb in chunks:
                    na, nb = len(la), len(lb)
                    # lane input tiles: [128 partitions, ngroups*512]
                    # input DMAs on two different engine rings (sync/gpsimd)
                    # so the lane tiles load in parallel
                    xta = xpool.tile([128, na * H], mybir.dt.bfloat16,
                                     name="xta", tag="xta")
                    nc.sync.dma_start(
                        out=xta[0:KW, :].rearrange("f (g y) -> f g y", g=na),
                        in_=xa[n, la[0]:la[0] + na].rearrange("g f y -> f g y"),
                    )
                    xtb = xpool.tile([128, nb * H], mybir.dt.bfloat16,
                                     name="xtb", tag="xtb")
                    nc.gpsimd.dma_start(
                        out=xtb[64:64 + KW, :].rearrange("f (g y) -> f g y", g=nb),
                        in_=xb[n, lb[0]:lb[0] + nb].rearrange("g f y -> f g y"),
                    )
                    # group stride padded to 512 so the DMA AP keeps its
                    # (group, y) structure (contiguous dims would merge and
                    # the DMA balancer cannot re-split them)
                    st = spool.tile([M, len(gl) * 512], mybir.dt.bfloat16,
                                    name="st", tag="st")
                    for k, g in enumerate(gl):
                        lane = g % 2
                        if lane == 0:
                            base, xt, idx = 0, xta, g // 2 - la[0]
                        else:
                            base, xt, idx = 64, xtb, g // 2 - lb[0]
                        ps = ppool.tile([MP, HO], mybir.dt.float32,
                                        name="ps", tag="ps")
                        for dy in range(FS):
                            nc.tensor.matmul(
                                ps[:, :],
                                wt[base:base + KW, dy * MP:(dy + 1) * MP],
                                xt[base:base + KW,
                                   idx * H + dy: idx * H + dy + HO],
                                start=(dy == 0), stop=(dy == FS - 1),
                            )
                        dst = st[:, k * 512:k * 512 + HO]
                        if lane == 0:
                            nc.scalar.activation(
                                dst, ps[0:M, :],
                                mybir.ActivationFunctionType.Identity,
                                bias=bt[:, :])
                        else:
                            nc.vector.tensor_scalar_add(dst, ps[0:M, :],
                                                        bt[:, :])
                    # output DMA: full groups in one burst-friendly write
                    # output DMAs on the scalar-engine ring, off the input rings
                    st3 = st.rearrange("p (g q) -> p g q", q=512)
                    x0 = P * gl[0]
                    nc.scalar.dma_start(
                        out=out[n, x0:x0 + P * len(gl), :, 0:HO]
                            .rearrange("(g x) c y -> (x c) g y", g=len(gl)),
                        in_=st3[:, 0:len(gl), 0:HO],
                    )
    if split_waits:
        _split_excess_waits(nc)
    return nc


def _prep_shared(weights3, weights4, weights4_4, weights6, bias1):
    Wd = _dense_kernel_np(np.asarray(weights3, np.float32),
                          np.asarray(weights4, np.float32),
                          np.asarray(weights4_4, np.float32),
                          np.asarray(weights6, np.float32))
    wb = _build_wbands(Wd)  # [5, 60, 96]
    w_flat = np.zeros((124, FS * MP), dtype=ml_dtypes.bfloat16)
    wcat = np.ascontiguousarray(
        wb.transpose(1, 0, 2).reshape(KW, FS * MP)).astype(ml_dtypes.bfloat16)
    w_flat[0:KW] = wcat       # lane A (partitions 0-59)
    w_flat[64:64 + KW] = wcat  # lane B (partitions 64-123)
    b_vec = np.ascontiguousarray(
        np.tile(np.asarray(bias1, np.float32), P)[:, None])
    return w_flat, b_vec


def _prep_windows(inputs):
    """xg[i, g, f, y] = xpad[i, y, 36g+f] as bf16, split into even/odd lanes."""
    nb = np.asarray(inputs, np.float32).shape[0]
    xin = np.asarray(inputs, np.float32).reshape(nb, H, FLAT)
    xpad = np.zeros((nb, H, FLAT_PAD), dtype=ml_dtypes.bfloat16)
    xpad[:, :, :FLAT] = xin.astype(ml_dtypes.bfloat16)
    s = xpad.strides
    xw = np.lib.stride_tricks.as_strided(
        xpad, shape=(nb, NG, KW, H), strides=(s[0], 36 * s[2], s[2], s[1]))
    xga = np.ascontiguousarray(xw[:, 0::2])  # [B, 43, 60, 512]
    xgb = np.ascontiguousarray(xw[:, 1::2])  # [B, 42, 60, 512]
    return xga, xgb


def run(inputs, weights3, weights4, weights4_4, weights6, bias1, trace=False):
    from concourse.bass_utils import run_bass_kernel_spmd

    if "nc" not in _CACHE:
        _CACHE["nc"] = _build_nc()
    nc = _CACHE["nc"]

    w_flat, b_vec = _prep_shared(weights3, weights4, weights4_4, weights6, bias1)
    xga, xgb = _prep_windows(inputs)

    in_maps = [
        {"xa": xga[c * IMGS_PER_CORE:(c + 1) * IMGS_PER_CORE],
         "xb": xgb[c * IMGS_PER_CORE:(c + 1) * IMGS_PER_CORE],
         "w": w_flat, "b": b_vec}
        for c in range(N_CORES)
    ]
    res = run_bass_kernel_spmd(nc, in_maps, core_ids=list(range(N_CORES)),
                               trace=trace)
    # device layout [img, x(510), c, y(512)] bf16 -> [B, y, x, c] f32
    out_t = np.concatenate([r["out"] for r in res.results], axis=0)
    out = np.ascontiguousarray(
        out_t[:, :WO, :, :HO].transpose(0, 3, 1, 2)).astype(np.float32)
    return out, res


def kernel(inputs, weights3, weights4, weights4_4, weights6, bias1):
    out, _ = run(inputs, weights3, weights4, weights4_4, weights6, bias1)
    return out
